# revision 72
# baseline (speedup 1.0000x reference)
"""Trainium2 Bass kernel for DeformAxialDW (v2: bf16 I/O, identity-folded).

out = x + convH(x) + convW(x), depthwise 7-tap convs along H/W with
fractional dilation r realized by bilinear sampling; expanded to integer-tap
banded (Toeplitz) convs with 2S+1 taps, S = floor(3r)+1.

v2 design (per core = one batch item, 8 cores):
  - x and out travel as bf16 in h-major DRAM layout [H, C, W] so each DMA
    descriptor moves G*W*2 = 3.5KB contiguous (full bus efficiency); the
    host does the fp32<->bf16 casts and [C,H,W]<->[H,C,W] transposes.
  - H-conv: per-channel [112+2S, 112] Toeplitz stationary WITH the identity
    (+x) folded in as a shifted unit diagonal; x blocks carry a 2S-row halo
    so no edge matmuls and no separate identity add are needed.
  - W-conv: PE-transpose 112x112 blocks of x (bf16 x^T stationary, fp8e4
    W-Toeplitz moving) accumulating into the same PSUM tile as the H-conv.
  - PSUM tiles hold channel PAIRS; the channel loop is software-pipelined
    (depth 2) because each engine queue retires in order; transpose drains
    ride DVE (2x 16-bit mode), output copies ride Act, loads ride the SP
    HWDGE queue and stores the Pool/SWDGE queue (GPSIMD cannot touch PSUM).
  - group plan 14x8 + 4x4 channels: the half-size tail groups shorten the
    end-of-pipeline drain.
"""

import sys

import numpy as np

sys.path.insert(0, "/opt/trn_rl_repo")

import ml_dtypes

BF16 = ml_dtypes.bfloat16
FP8 = ml_dtypes.float8_e4m3

C, H, W = 128, 224, 224
B = 8
HS = 112  # h/w block size

_CACHE = {}


def _tap_coeffs(w_taps: np.ndarray, r_val: float, S: int) -> np.ndarray:
    """Expand 7 fractional-dilation taps into 2S+1 integer-shift coeffs."""
    Cn, K = w_taps.shape
    P = K // 2
    alpha = np.zeros((Cn, 2 * S + 1), dtype=np.float64)
    for i in range(K):
        k_pos = i - P
        delta = np.float32(k_pos) * np.float32(r_val)
        d0 = int(np.floor(delta))
        frac = float(np.float32(delta) - np.float32(d0))
        alpha[:, d0 + S] += (1.0 - frac) * w_taps[:, i].astype(np.float64)
        alpha[:, d0 + 1 + S] += frac * w_taps[:, i].astype(np.float64)
    return alpha


def _banded(alpha: np.ndarray, rows: int, cols: int, diag_off: int, S: int):
    """M[i, c, jj] = alpha[c, (i - jj + diag_off) + S] where |i-jj+diag_off|<=S."""
    Cn = alpha.shape[0]
    out = np.zeros((rows, Cn, cols), dtype=np.float64)
    i = np.arange(rows)[:, None]
    jj = np.arange(cols)[None, :]
    d = i - jj + diag_off
    mask = np.abs(d) <= S
    ii, jjj = np.nonzero(mask)
    out[ii, :, jjj] = alpha[:, d[ii, jjj] + S].T
    return out


def _build_nc(S: int):
    import concourse.mybir as mybir
    from concourse import bacc
    from concourse.tile import TileContext

    f32 = mybir.dt.float32
    bf16 = mybir.dt.bfloat16
    fp8 = mybir.dt.float8e4

    HP = HS + 2 * S  # x block partitions (halo above and below)
    GW = HS + 3 * S  # W-Toeplitz band width

    nc = bacc.Bacc("TRN2", target_bir_lowering=False, debug=False)
    x_p = nc.declare_dram_parameter("x", [H, C, W], bf16, isOutput=False)
    th_p = nc.declare_dram_parameter("th", [HP, C, HS], bf16, isOutput=False)
    gw_p = nc.declare_dram_parameter("gw", [HS, C, GW], fp8, isOutput=False)
    id_p = nc.declare_dram_parameter("ident", [HS, HS], bf16, isOutput=False)
    z_p = nc.declare_dram_parameter("zeros", [S, 8, W], bf16, isOutput=False)
    out_p = nc.declare_dram_parameter("out", [H, C, W], bf16, isOutput=True)

    G = 8  # channels per DMA group
    with TileContext(nc) as tc:
        with tc.tile_pool(name="const", bufs=1) as constp, \
             tc.tile_pool(name="xb", bufs=4) as xbp, \
             tc.tile_pool(name="gt", bufs=4) as gtp, \
             tc.tile_pool(name="xt", bufs=6) as xtp, \
             tc.tile_pool(name="outs", bufs=6) as outp, \
             tc.tile_pool(name="pp", bufs=2, space="PSUM") as ppp, \
             tc.tile_pool(name="po", bufs=2, space="PSUM") as pop:
            ident = constp.tile([HS, HS], bf16)
            nc.gpsimd.dma_start(out=ident[:, :], in_=id_p[:, :])
            # PSUM-reading copies may only run on DVE/Act (GPSIMD cannot
            # access PSUM). bf16->bf16 transpose drains get DVE's 2x mode;
            # fp32 PSUM output copies lean on Act.
            def cp_xts(out, in_):
                nc.vector.tensor_copy(out=out, in_=in_)

            def cp(k, out, in_, spread=False):
                if spread and k % 2 == 0:
                    nc.vector.tensor_copy(out=out, in_=in_)
                else:
                    nc.scalar.copy(out=out, in_=in_)

            rr = 0
            pend = []  # channel pairs transposed, conv-chains not yet emitted

            def emit_chain():
                """H+W matmul chains + output copies/stores for one pair.

                Runs lagged (software pipelined) so the xts cast-copy of this
                pair finished while newer pairs' transposes kept the in-order
                PE queue busy.
                """
                nonlocal rr
                pr, gz, spread, last, c0_, xb_, thg_, gwg_, xts_, og_ = pend.pop(0)
                po_ = [None, None]
                for t in (0, 1):
                    po_[t] = pop.tile(
                        [HS, 2, W], f32, tag=f"po{t}", name=f"po{t}")
                for u in (0, 1):
                    cl = 2 * pr + u
                    for t in (0, 1):
                        # H-conv + identity (folded into th)
                        nc.tensor.matmul(
                            out=po_[t][:, u, :],
                            lhsT=thg_[0:HP, cl, :],
                            rhs=xb_[t][0:HP, cl, :],
                            start=True, stop=False,
                        )
                        # W-conv: two w_in chunks
                        nc.tensor.matmul(
                            out=po_[t][:, u, 0:HS + S],
                            lhsT=xts_[:, u, 0, t, :],
                            rhs=gwg_[0:HS, cl, 2 * S:3 * S + HS],
                            start=False, stop=False,
                        )
                        nc.tensor.matmul(
                            out=po_[t][:, u, HS - S:W],
                            lhsT=xts_[:, u, 1, t, :],
                            rhs=gwg_[0:HS, cl, S:2 * S + HS],
                            start=False, stop=True,
                        )
                # stores ride the Pool/SWDGE queue (keeps HWDGE free for
                # loads, Act free for copies); interleaved per t so the
                # store's descriptor generation overlaps the other copy
                for t in (0, 1):
                    cp(rr, og_[t][:, 2 * pr:2 * pr + 2, :], po_[t][:, :, :],
                       spread=spread)
                    rr += 1
                    if pr == gz // 2 - 1:
                        # the very last stores take the Act/HWDGE queue:
                        # shorter desc-gen on the kernel's critical end-path
                        dma_q = nc.scalar if last else nc.gpsimd
                        dma_q.dma_start(
                            out=out_p[t * HS:(t + 1) * HS, c0_:c0_ + gz, :],
                            in_=og_[t][:, :, :],
                        )

            # group plan: 14 groups of 8 channels, then 4 groups of 4 so
            # the end-of-pipeline lag drains in half-size steps
            plan = [(c0, G) for c0 in range(0, C - 2 * G, G)]
            plan += [(c0, G // 2) for c0 in range(C - 2 * G, C, G // 2)]
            zinit = {}
            for gi, (c0, gz) in enumerate(plan):
                # x block tiles: partitions [0,112) = rows [112t, 112t+112),
                # [112, 112+S) = above-halo rows, [112+S, 112+2S) = below-halo
                # rows (row order matched by host-permuted th rows). Halo
                # partitions that fall outside [0, H) stay zero: each pool
                # slot's dead strip is zero-filled on its first use.
                xb0 = xbp.tile([HP, gz, W], bf16, tag=f"xb0_{gz}")
                xb1 = xbp.tile([HP, gz, W], bf16, tag=f"xb1_{gz}")
                if zinit.get(gz, 0) < 4:
                    zinit[gz] = zinit.get(gz, 0) + 1
                    nc.gpsimd.dma_start(
                        out=xb0[HS + S:HP, :, :], in_=z_p[:, 0:gz, :])
                    nc.gpsimd.dma_start(
                        out=xb1[HS:HS + S, :, :], in_=z_p[:, 0:gz, :])
                if gi == 0:
                    # split the first loads so pair-0 transposes start a
                    # half-load earlier (ramp-in)
                    for h in (0, 1):
                        nc.sync.dma_start(
                            out=xb0[0:HS + S, 4 * h:4 * h + 4, :],
                            in_=x_p[0:HS + S, c0 + 4 * h:c0 + 4 * h + 4, :])
                        nc.sync.dma_start(
                            out=xb1[0:HS, 4 * h:4 * h + 4, :],
                            in_=x_p[HS:H, c0 + 4 * h:c0 + 4 * h + 4, :])
                else:
                    nc.sync.dma_start(
                        out=xb0[0:HS + S, :, :], in_=x_p[0:HS + S, c0:c0 + gz, :])
                    nc.sync.dma_start(
                        out=xb1[0:HS, :, :], in_=x_p[HS:H, c0:c0 + gz, :])
                nc.sync.dma_start(
                    out=xb1[HS + S:HP, :, :], in_=x_p[HS - S:HS, c0:c0 + gz, :])
                thg = gtp.tile([HP, gz, HS], bf16, tag=f"th_{gz}")
                gwg = gtp.tile([HS, gz, GW], fp8, tag=f"gw_{gz}")
                nc.sync.dma_start(out=thg[:, :, :], in_=th_p[:, c0:c0 + gz, :])
                nc.sync.dma_start(out=gwg[:, :, :], in_=gw_p[:, c0:c0 + gz, :])
                xb = [xb0, xb1]
                og0 = outp.tile([HS, gz, W], bf16, tag=f"ot0_{gz}")
                og1 = outp.tile([HS, gz, W], bf16, tag=f"ot1_{gz}")
                og = [og0, og1]
                spread = gi >= len(plan) - 4
                for pr in range(gz // 2):
                    # transpose both 112-wide w-chunks of both blocks for a
                    # channel pair; one cast-copy drains all 8 transposes
                    pp = ppp.tile([HS, 2, 2, 2, HS], bf16)
                    for u in (0, 1):
                        for q in (0, 1):
                            for t in (0, 1):
                                nc.tensor.matmul(
                                    out=pp[:, u, q, t, :],
                                    lhsT=xb[t][0:HS, 2 * pr + u,
                                               q * HS:(q + 1) * HS],
                                    rhs=ident[:, :],
                                    is_transpose=True,
                                    skip_group_check=True,
                                )
                    xts = xtp.tile([HS, 2, 2, 2, HS], bf16, tag="xt")
                    pend.append((pr, gz, spread, gi >= len(plan) - 2, c0, xb,
                                 thg, gwg, xts, og))
                    # shallower lag for the small tail groups: their chains
                    # drain sooner after the final loads
                    while len(pend) > (1 if gz == G // 2 else 2):
                        emit_chain()
                    # enqueue after the chain's output copies so those never
                    # wait behind this on the copy engines
                    cp_xts(xts[:, :, :, :, :], pp[:, :, :, :, :])
            while pend:
                emit_chain()
    nc.compile()
    return nc


def _prepare_consts(weight_h, weight_w, r):
    r_val = float(max(np.float32(r), np.float32(1.0)))
    S = int(np.floor(3.0 * r_val)) + 1
    assert S <= 8, f"dilation r={r_val} too large for this kernel (S={S})"
    HP = HS + 2 * S
    wh = np.asarray(weight_h)[:, 0, :, 0].astype(np.float64)
    ww = np.asarray(weight_w)[:, 0, 0, :].astype(np.float64)
    ah = _tap_coeffs(wh, r_val, S)
    aw = _tap_coeffs(ww, r_val, S)
    # th rows follow the xb tile's permuted row order: partition p holds the
    # x row at relative offset rel[p] from the block start, where
    # rel = [0..111, 112..112+S-1 (above-halo), -S..-1 (below-halo)].
    # th[p, c, j] = ah[c, rel[p]-j] band coeff, plus unit diagonal (the +x
    # identity) at rel[p] == j.
    rel = np.concatenate(
        [np.arange(HS), np.arange(HS, HS + S), np.arange(-S, 0)])
    d = rel[:, None] - np.arange(HS)[None, :]  # [HP, HS] tap offsets
    mask = np.abs(d) <= S
    th = np.zeros((HP, C, HS), dtype=np.float64)
    pp_, jj_ = np.nonzero(mask)
    th[pp_, :, jj_] = ah[:, d[pp_, jj_] + S].T
    th[np.arange(HS), :, np.arange(HS)] += 1.0
    th = th.astype(BF16)
    gw = _banded(aw, HS, HS + 3 * S, 2 * S, S).astype(FP8)
    ident = np.eye(HS, dtype=BF16)
    zeros = np.zeros((S, 8, W), dtype=BF16)
    return S, th, gw, ident, zeros


def kernel(x, weight_h, weight_w, r):
    from concourse.bass_utils import run_bass_kernel_spmd

    x = np.asarray(x, dtype=np.float32)
    assert x.shape == (B, C, H, W), x.shape
    S, th, gw, ident, zeros = _prepare_consts(weight_h, weight_w, r)

    if S not in _CACHE:
        _CACHE[S] = _build_nc(S)
    nc = _CACHE[S]

    # h-major bf16 input: [B, H, C, W]
    xh = np.ascontiguousarray(x.transpose(0, 2, 1, 3)).astype(BF16)
    in_maps = [
        {"x": xh[b], "th": th, "gw": gw, "ident": ident, "zeros": zeros}
        for b in range(B)
    ]
    res = run_bass_kernel_spmd(nc, in_maps, core_ids=list(range(B)))
    out = np.stack(
        [res.results[b]["out"].astype(np.float32).transpose(1, 0, 2)
         for b in range(B)],
        axis=0,
    )
    return out
